# revision 24
# baseline (speedup 1.0000x reference)
"""MultiHeadAttention Trainium2 kernel (8 NeuronCores).

Sharding: data-parallel over batch (2) x tensor-parallel over heads (16/4=4
head groups). Core c handles batch b = c//4 and heads 4g..4g+4 (g = c%4),
i.e. a 256-wide column slice of Wq/Wk/Wv and the matching row slice of Wo.
Each core computes a full [2048, 1024] partial output (its heads' ctx @ Wo
row-slice); the host sums the 4 partials per batch and adds the bias terms.

v3: fine-grained software pipeline. The host supplies x transposed, cast to
fp16 AND pre-tiled per 512-column chunk ([p, t, s] with 8KB contiguous per
partition) so DMA uses fat descriptors. Attention runs as a stream of
256-key groups per (q-chunk, head-pair) phase: 4 scores matmuls
(row-tile-paired on the PE) -> two N=1024 exps on ACT (double-buffered
2-bank PSUM tiles) -> ctx accumulation chains running ONE PHASE BEHIND
(so V-projection DMA has time), with QKV/out projections interleaved as
PE filler so both TensorE and ScalarE stay saturated from ~13us. Dummy
matmuls at t=0 warm the PE clock (HAM) during the initial DMA wait.
1/sqrt(dk) is folded into Wq/bq.

Per-core dataflow:
  Q.T, K.T = W.T @ xT + b (per-partition bias)     [d'=256, s] pair-packed
  V        = xT.T @ WvT (no bias; folded on host)  [s, c] + ones col/head
  scores.T = K.T_h.T @ Q.T_h (row-tile head pairs) [k, q] in PSUM
  P.T      = exp(scores.T) on ACT, fp16            [k, q] SBUF
  ctx.T|r  = [V_h | 1].T @ P.T (M=65, fused rowsum), 16-kt chains in PSUM
  ctx_n    = ctx.T * broadcast(1/r)                [c, q] pair-packed fp16
  out_u    = ctx_n.T @ WoT                         [s, 1024] -> DRAM f32
"""

import numpy as np

import concourse.bass as bass
import concourse.mybir as mybir
import concourse.tile as tile
from concourse import bacc
from concourse.bass_utils import run_bass_kernel_spmd

S = 2048          # sequence length
D = 1024          # model dim
DC = 256          # d' columns per core (4 heads x 64)
H = 4             # heads per core
DK = 64           # head dim
P = 128
F32 = mybir.dt.float32
FP16 = mybir.dt.float16
NCORES = 8
NCH = 4           # 512-column chunks of S

_cached = {}


def build_program():
    nc = bacc.Bacc("TRN2", target_bir_lowering=False, debug=False,
                   num_devices=NCORES)

    xq = [nc.dram_tensor(f"xq{i}", [P, 8, 512], FP16,
                         kind="ExternalInput").ap() for i in range(NCH)]
    xk = [nc.dram_tensor(f"xk{i}", [P, 8, 512], FP16,
                         kind="ExternalInput").ap() for i in range(NCH)]
    xv = [nc.dram_tensor(f"xv{i}", [P, 8, 512], FP16,
                         kind="ExternalInput").ap() for i in range(NCH)]
    wq = nc.dram_tensor("wq", [P, 8, DC], FP16, kind="ExternalInput").ap()
    wk = nc.dram_tensor("wk", [P, 8, DC], FP16, kind="ExternalInput").ap()
    wv = nc.dram_tensor("wv", [P, 8, DC], FP16, kind="ExternalInput").ap()
    wo = nc.dram_tensor("wo", [P, 2, D], FP16, kind="ExternalInput").ap()
    bqr = nc.dram_tensor("bqr", [P, 2], F32, kind="ExternalInput").ap()
    bkr = nc.dram_tensor("bkr", [P, 2], F32, kind="ExternalInput").ap()
    out = nc.dram_tensor("out", [S, D], F32, kind="ExternalOutput").ap()

    with tile.TileContext(nc) as tc:
        build_tile_kernel(nc, tc, xq, xk, xv, wq, wk, wv, wo, bqr, bkr, out)

    nc.compile()
    return nc


def build_tile_kernel(nc, tc, xq, xk, xv, wq, wk, wv, wo, bqr, bkr, out,
                      dbg=None):
    from contextlib import ExitStack

    with ExitStack() as ctx:
        singles = ctx.enter_context(tc.tile_pool(name="singles", bufs=1))
        persist = ctx.enter_context(tc.tile_pool(name="persist", bufs=1))
        # PSUM: 2x 2-bank score slots + 2 ctx accumulators + 2 general = 8
        psS = ctx.enter_context(tc.tile_pool(name="psS", bufs=2, space="PSUM"))
        psA = ctx.enter_context(tc.tile_pool(name="psA", bufs=2, space="PSUM"))
        psG = ctx.enter_context(tc.tile_pool(name="psG", bufs=2, space="PSUM"))
        xc_pool = ctx.enter_context(tc.tile_pool(name="xc", bufs=5))
        pT_pool = ctx.enter_context(tc.tile_pool(name="pT", bufs=2))
        norm_pool = ctx.enter_context(tc.tile_pool(name="norm", bufs=2))
        osb_pool = ctx.enter_context(tc.tile_pool(name="osb", bufs=2))

        # --- constants / weights (DMA priority order) ------------------------
        bq_t = singles.tile([P, 2], F32, tag="bq")
        bk_t = singles.tile([P, 2], F32, tag="bk")
        nc.sync.dma_start(out=bq_t, in_=bqr)
        nc.sync.dma_start(out=bk_t, in_=bkr)
        w_k = singles.tile([P, 8, DC], FP16, tag="w_k")
        w_q = singles.tile([P, 8, DC], FP16, tag="w_q")
        w_v = singles.tile([P, 8, DC], FP16, tag="w_v")
        w_o = singles.tile([P, 2, D], FP16, tag="w_o")
        nc.sync.dma_start(out=w_k, in_=wk)

        # --- persistent activations -----------------------------------------
        kT_c = [persist.tile([P, 2, 512], FP16, tag=f"kT{i}", name=f"kT{i}")
                for i in range(NCH)]
        qT_c = [persist.tile([P, 2, 512], FP16, tag=f"qT{i}", name=f"qT{i}")
                for i in range(NCH)]
        v_sb = persist.tile([P, 16, H * (DK + 1)], FP16, tag="v_sb")
        ctxn = [persist.tile([P, 2, 512], FP16, tag=f"ctxn{i}", name=f"ctxn{i}")
                for i in range(NCH)]

        # ACT table preload: tiny exp at t=0 so the ~2.7us table load
        # overlaps the initial DMA wait.
        wz = singles.tile([P, 2], F32, tag="wz")
        wa = singles.tile([P, 2], FP16, tag="wa")
        nc.vector.memset(wz, 0.0)
        nc.scalar.activation(wa, wz, mybir.ActivationFunctionType.Exp)

        for h in range(H):  # ones column per head for rowsum-in-matmul
            nc.vector.memset(v_sb[:, :, h * 65 + 64:h * 65 + 65], 1.0)

        # PE warm-up: keep TensorE busy during the initial DMA wait so the
        # HAM clock gate is at 2.4 GHz when real matmuls arrive (~60 N=256
        # dummies ~ 7us of coverage).
        dum = singles.tile([P, 256], FP16, tag="dum")
        dum512 = singles.tile([P, 512], FP16, tag="dum512")
        nc.vector.memset(dum, 0.0)
        nc.vector.memset(dum512, 0.0)
        dps = psG.tile([P, 256], F32, tag="psG", name="dum_ps")
        for i in range(60):
            nc.tensor.matmul(dps, lhsT=dum[:, 0:128], rhs=dum,
                             start=True, stop=True, skip_group_check=True)

        # --- emit helpers ----------------------------------------------------
        # qk-proj is split into two 8-MM filler units (one per m half)
        def emit_qk_proj_m(name, x_dram, w_t, b_t, dest, sc, m):
            if m == 0:
                xc = xc_pool.tile([P, 8, 512], FP16, tag="xc",
                                  name=f"xc_{name}{sc}")
                nc.sync.dma_start(out=xc, in_=x_dram[sc])
                emit_qk_proj_m.xc = xc
            xc = emit_qk_proj_m.xc
            pr = psG.tile([P, 512], F32, tag="psG",
                          name=f"pr_{name}_{sc}_{m}")
            for dt in range(8):
                nc.tensor.matmul(
                    pr,
                    lhsT=w_t[:, dt, 128 * m:128 * (m + 1)],
                    rhs=xc[:, dt, :],
                    start=(dt == 0), stop=(dt == 7))
            nc.vector.tensor_scalar_add(
                dest[sc][:, m, :], pr, b_t[:, m:m + 1])

        def emit_qk_proj(name, x_dram, w_t, b_t, dest, sc):
            for m in range(2):
                emit_qk_proj_m(name, x_dram, w_t, b_t, dest, sc, m)

        def emit_v_proj(sc, grp, i):
            # each 8-MM kt group is its own filler unit; two kt per PSUM bank
            if grp == 0 and i == 0:
                xc = xc_pool.tile([P, 8, 512], FP16, tag="xc",
                                  name=f"xc_v{sc}")
                nc.sync.dma_start(out=xc, in_=xv[sc])
                emit_v_proj.xc = xc
            if i == 0:
                emit_v_proj.pv = psG.tile([P, 2, DC], F32, tag="psG",
                                          name=f"pv_{sc}_{grp}")
            xc, pv = emit_v_proj.xc, emit_v_proj.pv
            ktl = 2 * grp + i
            for dt in range(8):
                nc.tensor.matmul(
                    pv[:, i, :],
                    lhsT=xc[:, dt, 128 * ktl:128 * (ktl + 1)],
                    rhs=w_v[:, dt, :],
                    start=(dt == 0), stop=(dt == 7),
                    skip_group_check=True)
            if i == 1:
                kt0 = 4 * sc + 2 * grp
                nc.vector.tensor_copy(
                    v_sb[:, kt0:kt0 + 2, :]
                        .rearrange("p k (h x) -> p k h x", h=H)[:, :, :, 0:DK],
                    pv.rearrange("p k (h c) -> p k h c", c=DK))

        def emit_scores_exp(qc, pr_i, g, pT):
            # 4 matmuls -> two 2-bank tiles (head a / head b); 2 exps
            sca = psS.tile([P, 2, 512], F32, tag="sc",
                           name=f"sca_{qc}_{pr_i}_{g}")
            scb = psS.tile([P, 2, 512], F32, tag="sc",
                           name=f"scb_{qc}_{pr_i}_{g}")
            for kti in range(2):
                kt = 2 * g + kti
                kc, ko = divmod(kt, 4)
                ksl = slice(128 * ko, 128 * (ko + 1))
                nc.tensor.matmul(sca[:, kti, :],
                                 lhsT=kT_c[kc][0:64, pr_i, ksl],
                                 rhs=qT_c[qc][0:64, pr_i, :])
                nc.tensor.matmul(scb[:, kti, :],
                                 lhsT=kT_c[kc][64:128, pr_i, ksl],
                                 rhs=qT_c[qc][64:128, pr_i, :])
            nc.scalar.activation(pT[:, 0, 2 * g:2 * g + 2, :], sca,
                                 mybir.ActivationFunctionType.Exp)
            nc.scalar.activation(pT[:, 1, 2 * g:2 * g + 2, :], scb,
                                 mybir.ActivationFunctionType.Exp)

        def emit_ctx(pr_i, kts, pT, accs):
            for kt in kts:
                for hp in range(2):
                    h = 2 * pr_i + hp
                    nc.tensor.matmul(
                        accs[hp][0:65, :],
                        lhsT=v_sb[:, kt, 65 * h:65 * h + 65],
                        rhs=pT[:, hp, kt, :],
                        start=(kt == 0), stop=(kt == 15))

        def emit_norm(qc, pr_i, accs):
            for hp in range(2):
                acc = accs[hp]
                rs = norm_pool.tile([1, 512], F32, tag="rs",
                                    name=f"rs_{qc}_{pr_i}_{hp}")
                # custom-DVE ops drop the input base partition, so stage the
                # rowsum row to partition 0 with a plain copy first
                nc.vector.tensor_copy(rs, acc[64:65, :])
                rc1 = norm_pool.tile([1, 512], F32, tag="rc1",
                                     name=f"rc1_{qc}_{pr_i}_{hp}")
                nc.vector.reciprocal_approx_fast(rc1, rs)
                bc = norm_pool.tile([64, 512], F32, tag="bc",
                                    name=f"bc_{qc}_{pr_i}_{hp}")
                nc.gpsimd.partition_broadcast(bc, rc1[0:1, :], channels=64)
                nc.vector.tensor_mul(
                    ctxn[qc][64 * hp:64 * hp + 64, pr_i, :],
                    acc[0:64, :], bc)

        def emit_outproj_st(qc, stl, tail=False):
            st = 4 * qc + stl
            ob = osb_pool.tile([P, D], F32, tag="ob", name=f"ob_{st}")
            for jc in range(2):
                op = psG.tile([P, 512], F32, tag="psG",
                              name=f"op_{st}_{jc}")
                for ct in range(2):
                    nc.tensor.matmul(
                        op,
                        lhsT=ctxn[qc][:, ct, 128 * stl:128 * (stl + 1)],
                        rhs=w_o[:, ct, 512 * jc:512 * (jc + 1)],
                        start=(ct == 0), stop=(ct == 1))
                if tail and jc == 0:
                    # no exps left: the scalar engine is free, split the
                    # copies across ACT and DVE
                    nc.scalar.copy(ob[:, 0:512], op)
                else:
                    nc.vector.tensor_copy(ob[:, 512 * jc:512 * (jc + 1)], op)
                # ship each half as soon as its copy lands
                nc.sync.dma_start(
                    out=out[128 * st:128 * (st + 1),
                            512 * jc:512 * (jc + 1)],
                    in_=ob[:, 512 * jc:512 * (jc + 1)])

        # --- filler queue (PE work interleaved during the exp grind; ordered
        # by when their DMA dependencies land; ~1us units) --------------------
        def qk_unit(name, x_dram, w_t, b_t, dest, sc):
            return [lambda m=m: emit_qk_proj_m(name, x_dram, w_t, b_t,
                                               dest, sc, m) for m in range(2)]

        def v_unit(sc, grp):
            return [lambda i=i: emit_v_proj(sc, grp, i) for i in range(2)]

        fillers = (
            qk_unit("k", xk, w_k, bk_t, kT_c, 1)
            + qk_unit("k", xk, w_k, bk_t, kT_c, 2)
            + qk_unit("k", xk, w_k, bk_t, kT_c, 3)
            + v_unit(0, 0) + v_unit(0, 1)
            + qk_unit("q", xq, w_q, bq_t, qT_c, 1)
            + v_unit(1, 0) + v_unit(1, 1)
            + v_unit(2, 0)
            + qk_unit("q", xq, w_q, bq_t, qT_c, 2)
            + v_unit(2, 1) + v_unit(3, 0) + v_unit(3, 1)
            + qk_unit("q", xq, w_q, bq_t, qT_c, 3)
        )

        ka_n = [0]

        def pop_filler(accs=None):
            if fillers:
                fillers.pop(0)()
            elif accs is not None:
                # HAM keepalive: no filler work this group, so the PE would
                # idle while ACT grinds exp; issue dummy matmuls on a free
                # psG bank (the proj/outproj pool is idle exactly when the
                # filler queue is empty).
                ka_n[0] += 1
                ka = psG.tile([P, 512], F32, tag="psG",
                              name=f"ka_{ka_n[0]}")
                for i in range(2):
                    nc.tensor.matmul(ka, lhsT=dum512[:, 0:128], rhs=dum512,
                                     start=True, stop=True,
                                     skip_group_check=True)

        # --- main pipeline ---------------------------------------------------
        emit_qk_proj("k", xk, w_k, bk_t, kT_c, 0)
        nc.sync.dma_start(out=w_q, in_=wq)
        emit_qk_proj("q", xq, w_q, bq_t, qT_c, 0)
        nc.sync.dma_start(out=w_v, in_=wv)
        nc.sync.dma_start(out=w_o, in_=wo)

        prev = None  # (qc, pr_i, pT, accs)
        for qc in range(NCH):
            for pr_i in range(2):
                pT = pT_pool.tile([P, 2, 16, 512], FP16, tag="pT",
                                  name=f"pT_{qc}_{pr_i}")
                if prev is not None:
                    p_qc, p_pr, p_pT = prev
                    accs = [psA.tile([P, 512], F32, tag="acc",
                                     name=f"acc_{p_qc}_{p_pr}_{hp}")
                            for hp in range(2)]
                # front-loaded one-behind ctx: finish by g5 so the serial
                # norm chain (copy->recip->broadcast->mul) overlaps g6-g7
                # instead of the phase boundary
                CTX_KTS = [(0, 1, 2), (3, 4, 5), (6, 7, 8), (9, 10, 11),
                           (12, 13), (14, 15), (), ()]
                for g in range(8):
                    emit_scores_exp(qc, pr_i, g, pT)
                    if prev is not None:
                        emit_ctx(p_pr, CTX_KTS[g], p_pT, accs)
                    # During the first two phases drain the projection queue
                    # at 2 units/group so every v-chunk is emitted before the
                    # ctx chain that consumes it (program order = PE order).
                    if qc == 0:
                        pop_filler()
                        pop_filler()
                    elif len(CTX_KTS[g]) < 3 or fillers:
                        pop_filler(accs if prev is not None else None)
                    if g == 5 and prev is not None:
                        emit_norm(p_qc, p_pr, accs)
                if prev is not None and p_pr == 1 and p_qc < 2:
                    for stl in range(4):
                        fillers.append(
                            lambda q=p_qc, s=stl: emit_outproj_st(q, s))
                prev = (qc, pr_i, pT)

        # --- tail: ctx burst (PE-dense, keeps HAM warm), then norm(3,1) on
        # DVE/GpSimd overlapped with outproj(2) matmuls, then outproj(3) -----
        p_qc, p_pr, p_pT = prev
        accs = [psA.tile([P, 512], F32, tag="acc",
                         name=f"acc_{p_qc}_{p_pr}_{hp}")
                for hp in range(2)]
        emit_ctx(p_pr, range(16), p_pT, accs)
        emit_norm(p_qc, p_pr, accs)
        for stl in range(4):
            emit_outproj_st(2, stl, tail=True)
        for stl in range(4):
            emit_outproj_st(3, stl, tail=True)
        while fillers:
            pop_filler()

        if dbg is not None:
            nc.sync.dma_start(out=dbg["kT"], in_=kT_c[0])
            nc.sync.dma_start(out=dbg["qT"], in_=qT_c[0])
            nc.sync.dma_start(out=dbg["v"], in_=v_sb)
            nc.sync.dma_start(out=dbg["pT"], in_=p_pT)
            nc.sync.dma_start(out=dbg["cn"], in_=ctxn[0])


def make_in_maps(Q_input, K_input, V_input, Wq, bq, Wk, bk, Wv, Wo):
    scale = 0.125  # 1/sqrt(64), exact power of two

    def tile_x(x):  # [S, D] f32 -> chunk-tiled x.T fp16 [4][128, 8, 512]
        xt = np.ascontiguousarray(x.T).astype(np.float16)      # [D, S]
        t4 = np.ascontiguousarray(
            xt.reshape(8, P, NCH, 512).transpose(2, 1, 0, 3))  # [sc,p,t,s]
        return [t4[i] for i in range(NCH)]

    def tile_w(wt):  # [D, DC] -> [128, 8, DC] fp16
        return np.ascontiguousarray(
            wt.reshape(8, P, DC).transpose(1, 0, 2)).astype(np.float16)

    xt = {}
    for b in range(2):
        xt[("q", b)] = tile_x(Q_input[b])
        xt[("k", b)] = tile_x(K_input[b])
        xt[("v", b)] = tile_x(V_input[b])

    in_maps = []
    for c in range(NCORES):
        b, g = divmod(c, 4)
        sl = slice(DC * g, DC * (g + 1))
        wot = np.ascontiguousarray(Wo[:, sl].T).astype(np.float16)  # [DC, D]
        m = {
            "wq": tile_w(np.ascontiguousarray(Wq[sl, :].T) * scale),
            "wk": tile_w(np.ascontiguousarray(Wk[sl, :].T)),
            "wv": tile_w(np.ascontiguousarray(Wv[sl, :].T)),
            "wo": np.ascontiguousarray(
                wot.reshape(2, P, D).transpose(1, 0, 2)),
            "bqr": np.ascontiguousarray(
                (bq[sl] * scale).reshape(2, P).T).astype(np.float32),
            "bkr": np.ascontiguousarray(
                bk[sl].reshape(2, P).T).astype(np.float32),
        }
        for i in range(NCH):
            m[f"xq{i}"] = xt[("q", b)][i]
            m[f"xk{i}"] = xt[("k", b)][i]
            m[f"xv{i}"] = xt[("v", b)][i]
        in_maps.append(m)
    return in_maps


def kernel(Q_input, K_input, V_input, Wq, bq, Wk, bk, Wv, bv, Wo, bo):
    if "nc" not in _cached:
        _cached["nc"] = build_program()
    nc = _cached["nc"]

    in_maps = make_in_maps(Q_input, K_input, V_input, Wq, bq, Wk, bk, Wv, Wo)
    res = run_bass_kernel_spmd(nc, in_maps, list(range(NCORES))).results
    outs = [res[c]["out"] for c in range(NCORES)]

    const = (bv.astype(np.float32) @ Wo.T.astype(np.float32)) + bo
    full = np.empty((2, S, D), np.float32)
    for b in range(2):
        acc = outs[4 * b].astype(np.float32).copy()
        for g in range(1, 4):
            acc += outs[4 * b + g]
        full[b] = acc + const
    return full
